# revision 67
# baseline (speedup 1.0000x reference)
"""Based-attention (Taylor linear attention + sliding window) TRN2 kernel.

Math: phi(u) = [1, u, outer(u,u)*sqrt(1/2)] satisfies
    phi(q) . phi(k) = 1 + q.k + 0.5*(q.k)^2
so causal linear attention with Taylor features is an ordinary causal
attention with elementwise weights A = 0.5*(G+1)^2 + 0.5, G = Q @ K^T.
The sliding-window softmax reuses the same G (scores are raw q.k).

All-f16 datapath. A ones-feature is appended to q/k (zero weight column
+ bias 1.0) so the PE computes G' = G+1 directly:
  A      = 0.5*G'^2 (+0.5 via rank-1 cumsum init / diag mask)
  window = exp(G'-7): the constant e^-6 cancels in the softmax ratio
           and keeps f16 in range on both ends.
Window masks are folded into PSUM with identity-matmuls (-60000 adds)
so exp needs no separate mask multiply.

Sharding: H=16 heads over 8 cores (2 heads/core).
"""

import sys

import numpy as np

sys.path.insert(0, "/opt/trn_rl_repo")

from concourse import bacc, mybir, tile  # noqa: E402
from concourse.bass_utils import run_bass_kernel_spmd  # noqa: E402

N = 1024
D = 1024
H = 16
DP = 16
DH = 64
W = 64
NCORES = 8
HPC = H // NCORES  # heads per core = 2
SQRT_HALF = float(1.0 / np.sqrt(2.0))
EXP_BIAS = -7.0  # exp(G' - 7) = exp(G - 6); cancels in the softmax ratio
NEG = -60000.0  # f16-safe "minus infinity" for window masking

F32 = mybir.dt.float32
F16 = mybir.dt.float16

KT = D // 128  # 8 contraction tiles
NCH = N // 128  # 8 token chunks
GRP = 4  # query chunks per group
NG = NCH // GRP

_CACHE = {}


def _emit(tc, nc, t):
    AluAdd = mybir.AluOpType.add
    AluMult = mybir.AluOpType.mult
    Act = mybir.ActivationFunctionType

    from contextlib import ExitStack

    with ExitStack() as ctx:
        cp = ctx.enter_context(tc.tile_pool(name="consts", bufs=1))

        # ---- input DMAs ----
        # all weights + masks + the qk bias row packed into ONE wide f16
        # tensor (big DMA lines); x tiles on the sync + scalar hwdge queues.
        # per-k: wqk(128: q0 0:17, q1 32:49, k0 64:81, k1 96:113) wv(128);
        # then ident/mlin/mdiag/moff masks; then the qk bias row (row 0).
        WCOLS = KT * 256 + 512 + 128
        mo = KT * 256
        bw = cp.tile([128, WCOLS], F16, tag="bw", name="bw")
        # per-k weight chunks so matmul k can start as soon as its slice lands
        for k in range(KT):
            nc.scalar.dma_start(
                bw[:, 256 * k : 256 * k + 256], t["bw"][:, 256 * k : 256 * k + 256]
            )
        nc.scalar.dma_start(bw[:, mo:WCOLS], t["bw"][:, mo:WCOLS])
        wqk = [bw[:, 256 * k : 256 * k + 128] for k in range(KT)]
        wv = [bw[:, 256 * k + 128 : 256 * k + 256] for k in range(KT)]
        ident = bw[:, mo : mo + 128]
        mlin = bw[:, mo + 128 : mo + 256]
        mdiag = bw[:, mo + 256 : mo + 384]
        moff = bw[:, mo + 384 : mo + 512]
        mboth = bw[:, mo + 256 : mo + 512]  # mdiag|moff for the fused mask-mm
        bias_row = bw[0:1, mo + 512 : mo + 640]

        xt = []
        xt_eng = [nc.sync, nc.gpsimd, nc.sync, nc.gpsimd, nc.sync, nc.gpsimd, nc.sync, nc.gpsimd]
        for k in range(KT):
            tl = cp.tile([128, N], F16, tag=f"xt{k}", name=f"xt{k}")
            xt_eng[k].dma_start(tl[:], t["xT"][128 * k : 128 * k + 128, :])
            xt.append(tl)

        bv_sb = cp.tile([128, 1], F32, tag="bv", name="bv")
        nc.scalar.dma_start(bv_sb[:], t["bv"][:, :])

        halfc = cp.tile([128, 1], F16, tag="halfc", name="halfc")
        nc.gpsimd.memset(halfc[:], 0.5)
        ebias = cp.tile([128, 1], F32, tag="ebias", name="ebias")
        nc.gpsimd.memset(ebias[:], EXP_BIAS)
        ones_row = cp.tile([1, 128], F16, tag="ones_row", name="ones_row")
        nc.gpsimd.memset(ones_row[:], 1.0)
        ones_x = cp.tile([1, N], F16, tag="ones_x", name="ones_x")
        nc.gpsimd.memset(ones_x[:], 1.0)

        # merged q/k tile: head lh has q at rows 64*lh..64*lh+17 and k at
        # rows 64*lh+32..64*lh+49; kt_sb gets the k rows DMA-shifted down 32
        # so lhsT/rhs of the G-matmul share a partition base (64*lh).
        qt_sb = cp.tile([128, N], F16, tag="qt", name="qt")
        kt_sb = cp.tile([128, N], F16, tag="kt", name="kt")
        vt_sb = cp.tile([128, N], F16, tag="vt", name="vt")
        # token-major V: cols 0:64 h0-v, 64 ones, 65:129 h1-v, 129 ones
        vcat = [cp.tile([128, 130], F16, tag=f"vch{j}", name=f"vch{j}") for j in range(NCH)]
        for j in range(NCH):
            nc.gpsimd.memset(vcat[j][:, 64:65], 1.0)
            nc.gpsimd.memset(vcat[j][:, 129:130], 1.0)
        # ci[i] = 0.5 * cumulative colsum of vcat chunks 0..i  (f16 for matmul rhs)
        ci = [cp.tile([1, 130], F16, tag=f"ci{i}", name=f"ci{i}") for i in range(NCH)]

        # ---- phase A: projections + V transposes + ci, interleaved so the
        # PE never waits on the scalar/vector extractions ----
        with tc.tile_pool(name="psA", bufs=2, space="PSUM") as psA, tc.tile_pool(
            name="psT", bufs=2, space="PSUM"
        ) as psT:

            def emit_tr(j):
                pst = psT.tile([128, 128], F16, tag="vtr", name="vtr")
                nc.tensor.transpose(pst[:], vt_sb[:, 128 * j : 128 * j + 128], ident)
                # both 64-col halves in one strided copy
                dst = vcat[j][:, 0:130].rearrange("p (a b) -> p a b", b=65)[:, :, 0:64]
                src = pst[:, 0:128].rearrange("p (a b) -> p a b", b=64)
                nc.vector.tensor_copy(dst, src)

            def emit_half(h2, inject=None):
                s = slice(512 * h2, 512 * h2 + 512)
                psv = psA.tile([128, 512], F32, tag="psv", name="psv")
                psqk = psA.tile([128, 512], F32, tag="psqk", name="psqk")
                for k in range(KT):
                    nc.tensor.matmul(
                        psv[:], wv[k], xt[k][:, s],
                        start=(k == 0), stop=(k == KT - 1),
                    )
                    nc.tensor.matmul(
                        psqk[:], wqk[k], xt[k][:, s],
                        start=(k == 0), stop=False,
                    )
                    if inject is not None and k == 1:
                        for j in inject:
                            emit_tr(j)
                # rank-1 ones row: adds the q/k biases and the ones feature
                nc.tensor.matmul(
                    psqk[:], bias_row, ones_x[0:1, s], start=False, stop=True,
                )
                nc.scalar.activation(vt_sb[:, s], psv[:], Act.Identity, bias=bv_sb[:])
                nc.scalar.activation(qt_sb[:, s], psqk[:], Act.Identity)
                # k rows {32:49, 96:113} -> kt rows {0:17, 64:81}
                nc.sync.dma_start(kt_sb[0:17, s], qt_sb[32:49, s])
                nc.sync.dma_start(kt_sb[64:81, s], qt_sb[96:113, s])

            def emit_ci(j):
                psc = psT.tile([1, 130], F32, tag="psc", name="psc")
                nc.tensor.matmul(psc[:], halfc[:], vcat[j][:], start=True, stop=True)
                if j == 0:
                    nc.vector.tensor_copy(ci[0][:], psc[:])
                else:
                    nc.vector.tensor_add(ci[j][:], ci[j - 1][:], psc[:])

            emit_half(0)
            emit_half(1, inject=range(4))
            for j in range(4):
                emit_ci(j)
            for j in range(4, NCH):
                emit_tr(j)
            for j in range(4, NCH):
                emit_ci(j)

        if "dbg_qt" in t:
            nc.sync.dma_start(t["dbg_qt"][:, :], qt_sb[:])
            nc.sync.dma_start(t["dbg_kt"][:, :], kt_sb[:])
            nc.sync.dma_start(t["dbg_vt"][:, :], vt_sb[:])
            nc.sync.dma_start(t["dbg_vc0"][:, :], vcat[0][:])
            nc.sync.dma_start(t["dbg_ci"][:, :], ci[6][:])

        # ---- phase B: attention ----
        psg = ctx.enter_context(tc.tile_pool(name="psg", bufs=3, space="PSUM"))
        psy = ctx.enter_context(tc.tile_pool(name="psy", bufs=1, space="PSUM"))
        sba = ctx.enter_context(tc.tile_pool(name="sba", bufs=3))
        sbe = ctx.enter_context(tc.tile_pool(name="sbe", bufs=3))
        sbf = ctx.enter_context(tc.tile_pool(name="sbf", bufs=3))

        a_rr = [0]  # round-robin ACT/DVE split for the a-extraction

        def make_group(lh, g):
            r17 = slice(64 * lh, 64 * lh + 17)
            vsl = slice(65 * lh, 65 * lh + 65)
            i0, i1 = GRP * g, GRP * g + GRP
            m0, m1 = 128 * i0, 128 * i1
            ys = []
            stash = {}

            if True:

                def emit_g(j):
                    mstart = max(128 * j, m0)
                    span = m1 - mstart
                    wlo = max(128 * j, m0)
                    whi = min(128 * (j + 2), m1)
                    has_win = whi > wlo
                    pg = psg.tile([128, span], F32, tag="g", name="g")
                    nc.tensor.matmul(
                        pg[:], kt_sb[r17, 128 * j : 128 * j + 128],
                        qt_sb[r17, mstart:m1],
                        start=True, stop=not has_win,
                    )
                    # a = 0.5*G'^2  (alternate ACT / DVE)
                    a = sba.tile([128, span], F16, tag="a", name="a")
                    if a_rr[0] % 3 != 2:
                        nc.scalar.activation(a[:], pg[:], Act.Square, scale=SQRT_HALF)
                    else:
                        w = sba.tile([128, span], F16, tag="w", name="w")
                        nc.vector.tensor_copy(w[:], pg[:])
                        nc.vector.scalar_tensor_tensor(
                            a[:], w[:], 0.5, w[:], AluMult, AluMult
                        )
                    a_rr[0] += 1
                    if j >= i0:
                        dc = 128 * j - mstart
                        nc.vector.scalar_tensor_tensor(
                            a[:, dc : dc + 128], a[:, dc : dc + 128], 0.5, mlin,
                            AluAdd, AluMult,
                        )
                    stash[j] = (mstart, span, pg, a, wlo, whi, has_win)

                def emit_post(j):
                    mstart, span, pg, a, wlo, whi, has_win = stash.pop(j)
                    e = None
                    if has_win:
                        # fold window masks into PSUM after `a` has read it
                        both = wlo == 128 * j and whi == 128 * (j + 2)
                        if both:
                            dc = 128 * j - mstart
                            nc.tensor.matmul(
                                pg[:, dc : dc + 256], ident, mboth,
                                start=False, stop=True, skip_group_check=True,
                            )
                        elif wlo == 128 * j:
                            dc = 128 * j - mstart
                            nc.tensor.matmul(
                                pg[:, dc : dc + 128], ident, mdiag,
                                start=False, stop=True, skip_group_check=True,
                            )
                        else:
                            oc = 128 * (j + 1) - mstart
                            nc.tensor.matmul(
                                pg[:, oc : oc + 128], ident, moff,
                                start=False, stop=True, skip_group_check=True,
                            )
                        e = sbe.tile([128, whi - wlo], F16, tag="e", name="e")
                        nc.scalar.activation(
                            e[:], pg[:, wlo - mstart : whi - mstart], Act.Exp,
                            bias=ebias[:],
                        )
                    for i in range(max(j, i0), i1):
                        ic = 128 * i - mstart
                        nc.tensor.matmul(
                            ys[i - i0][:, 0:65], a[:, ic : ic + 128], vcat[j][:, vsl],
                            start=(j == 0 and i == 0), stop=(j == i),
                        )
                    if e is not None:
                        if wlo == 128 * j:
                            nc.tensor.matmul(
                                ys[j - i0][:, 65:130], e[:, 0:128], vcat[j][:, vsl],
                                start=False, stop=True, skip_group_check=True,
                            )
                        if whi == 128 * (j + 2) and i0 <= j + 1 < i1:
                            ec = 128 * (j + 1) - wlo
                            nc.tensor.matmul(
                                ys[j + 1 - i0][:, 65:130], e[:, ec : ec + 128],
                                vcat[j][:, vsl],
                                start=False, stop=False, skip_group_check=True,
                            )
                    # ys[q] complete after its diag e-mm: full per-i finals
                    if j >= i0:
                        q = j - i0
                        i = j
                        ysb = sbf.tile([128, 130], F16, tag="ysb", name="ysb")
                        if q % 2 == 0:
                            nc.scalar.activation(ysb[:], ys[q][:], Act.Identity)
                        else:
                            nc.vector.tensor_copy(ysb[:], ys[q][:])
                        rr = sbf.tile([128, 2], F32, tag="rr", name="rr")
                        dens = ysb[:, 0:130].rearrange("p (a b) -> p a b", b=65)[:, :, 64]
                        nc.vector.reciprocal(rr[:], dens)
                        t1 = sbf.tile([128, 64], F16, tag="t1", name="t1")
                        if q % 2 == 0:
                            nc.scalar.activation(
                                t1[:], ysb[:, 0:64], Act.Identity, scale=rr[:, 0:1]
                            )
                        else:
                            nc.vector.tensor_scalar_mul(t1[:], ysb[:, 0:64], rr[:, 0:1])
                        yo = sbf.tile([128, 64], F16, tag="yo", name="yo")
                        nc.vector.scalar_tensor_tensor(
                            yo[:], ysb[:, 65:129], rr[:, 1:2], t1[:], AluMult, AluAdd,
                        )
                        nc.sync.dma_start(
                            t[f"y{lh}"][128 * i : 128 * i + 128, :], yo[:],
                        )
                        if "dbg_ys0" in t and lh == 0 and g == 0 and q == 0:
                            nc.sync.dma_start(t["dbg_ys0"][:, :], ysb[:])
                    if "dbg_a" in t and lh == 0 and g == 0 and j == 0:
                        nc.sync.dma_start(t["dbg_a"][:, 0:span], a[:])
                        if e is not None:
                            nc.sync.dma_start(t["dbg_e"][:, 0 : whi - wlo], e[:])

                def head():
                    emit_g(0)
                    emit_g(1)

                def body():
                    ys.extend(
                        psy.tile([128, 130], F32, tag=f"yw{i - i0}", name=f"yw{i - i0}")
                        for i in range(i0, i1)
                    )
                    for i in range(i0, i1):
                        if i > 0:
                            nc.tensor.matmul(
                                ys[i - i0][:, 0:65], ones_row[:], ci[i - 1][0:1, vsl],
                                start=True, stop=False,
                            )
                    for j in range(2, i1):
                        emit_g(j)
                        emit_post(j - 2)
                    emit_post(i1 - 2)
                    emit_post(i1 - 1)

                return head, body

        groups = [make_group(lh, g) for lh in range(HPC) for g in range(NG)]
        for head, body in groups:
            head()
            body()


def _build(dbg=False):
    key = ("nc", dbg)
    if key in _CACHE:
        return _CACHE[key]
    nc = bacc.Bacc("TRN2", target_bir_lowering=False, debug=False)
    t = {
        "xT": nc.dram_tensor("xT", [D, N], F16, kind="ExternalInput").ap(),
        "bw": nc.dram_tensor("bw", [128, KT * 256 + 640], F16, kind="ExternalInput").ap(),
        "bv": nc.dram_tensor("bv", [128, 1], F32, kind="ExternalInput").ap(),
        "y0": nc.dram_tensor("y0", [N, DH], F16, kind="ExternalOutput").ap(),
        "y1": nc.dram_tensor("y1", [N, DH], F16, kind="ExternalOutput").ap(),
    }
    if dbg:
        t["dbg_qt"] = nc.dram_tensor("dbg_qt", [128, N], F16, kind="ExternalOutput").ap()
        t["dbg_kt"] = nc.dram_tensor("dbg_kt", [128, N], F16, kind="ExternalOutput").ap()
        t["dbg_vt"] = nc.dram_tensor("dbg_vt", [128, N], F16, kind="ExternalOutput").ap()
        t["dbg_vc0"] = nc.dram_tensor("dbg_vc0", [128, 130], F16, kind="ExternalOutput").ap()
        t["dbg_ci"] = nc.dram_tensor("dbg_ci", [1, 130], F16, kind="ExternalOutput").ap()
        t["dbg_a"] = nc.dram_tensor("dbg_a", [128, 512], F16, kind="ExternalOutput").ap()
        t["dbg_e"] = nc.dram_tensor("dbg_e", [128, 256], F16, kind="ExternalOutput").ap()
        t["dbg_ys0"] = nc.dram_tensor("dbg_ys0", [128, 130], F16, kind="ExternalOutput").ap()
    with tile.TileContext(nc) as tc:
        _emit(tc, nc, t)
    nc.compile()
    _CACHE[key] = nc
    return nc


def _masks():
    n = np.arange(128)[:, None]  # key within chunk
    m = np.arange(128)[None, :]  # query within chunk
    mlin = (n <= m).astype(np.float16)
    mdiag = np.where((m - n >= 0) & (m - n <= W - 1), 0.0, NEG).astype(np.float16)
    moff = np.where(n - m >= W + 1, 0.0, NEG).astype(np.float16)
    return mlin, mdiag, moff


def _in_maps(x, Wq, bq, Wk, bk, Wv, bv):
    xs = np.asarray(x, np.float32)[0]  # [N, D]
    xT = np.ascontiguousarray(xs.T).astype(np.float16)
    mlin, mdiag, moff = _masks()
    ident = np.eye(128, dtype=np.float16)
    Wq = np.asarray(Wq, np.float32).reshape(H, DP, D)
    Wk = np.asarray(Wk, np.float32).reshape(H, DP, D)
    Wv = np.asarray(Wv, np.float32).reshape(H, DH, D)
    bq = np.asarray(bq, np.float32).reshape(H, DP)
    bk = np.asarray(bk, np.float32).reshape(H, DP)
    bv = np.asarray(bv, np.float32).reshape(H, DH)
    maps = []
    for c in range(NCORES):
        h0, h1 = HPC * c, HPC * c + 1
        # qk weights: head li has q at 64*li..+16 and k at 64*li+32..+48;
        # biases + the ones feature (rows 16/48/80/112) ride a rank-1 ones
        # matmul via bias_row.
        wqk2 = np.zeros((D, 128), np.float32)
        brow = np.zeros((1, 128), np.float32)
        for li, h in enumerate((h0, h1)):
            o = 64 * li
            wqk2[:, o : o + 16] = Wq[h].T
            wqk2[:, o + 32 : o + 48] = Wk[h].T
            brow[0, o : o + 16] = bq[h]
            brow[0, o + 16] = 1.0
            brow[0, o + 32 : o + 48] = bk[h]
            brow[0, o + 48] = 1.0
        wv2 = np.concatenate([Wv[h0].T, Wv[h1].T], axis=1).astype(np.float16)
        bv2 = np.concatenate([bv[h0], bv[h1]])[:, None]
        bw = np.zeros((128, KT * 256 + 640), np.float16)
        for k in range(KT):
            r = slice(128 * k, 128 * k + 128)
            bw[:, 256 * k : 256 * k + 128] = wqk2[r]
            bw[:, 256 * k + 128 : 256 * k + 256] = wv2[r]
        mo = KT * 256
        bw[:, mo : mo + 128] = ident
        bw[:, mo + 128 : mo + 256] = mlin
        bw[:, mo + 256 : mo + 384] = mdiag
        bw[:, mo + 384 : mo + 512] = moff
        bw[0, mo + 512 : mo + 640] = brow[0]
        maps.append(
            {
                "xT": xT,
                "bw": bw,
                "bv": bv2.astype(np.float32),
            }
        )
    return maps


def _ensure_ntff_hook():
    """The agent image's antenv lacks axon_hooks; shim it so trace=True
    (NTFF profiling) works through bass_utils under axon."""
    import types

    try:
        import antenv.axon_hooks  # noqa: F401

        return
    except ImportError:
        pass
    try:
        import antenv
        from trn_agent_boot.trn_boot import _ntff_profile_via_ctypes

        hook = _ntff_profile_via_ctypes("/opt/axon/libaxon_pjrt.so")
        mod = types.ModuleType("antenv.axon_hooks")
        mod.get_axon_ntff_profile_hook = lambda: hook
        mod.set_axon_ntff_profile_hook = lambda h: None
        sys.modules["antenv.axon_hooks"] = mod
        antenv.axon_hooks = mod
    except Exception:
        pass


def _run(in_maps, trace=False, dbg=False):
    nc = _build(dbg)
    if trace:
        _ensure_ntff_hook()
    return run_bass_kernel_spmd(nc, in_maps, list(range(NCORES)), trace=trace)


def debug_run(x, Wq, bq, Wk, bk, Wv, bv):
    return _run(_in_maps(x, Wq, bq, Wk, bk, Wv, bv), dbg=True)


def kernel(x, Wq, bq, Wk, bk, Wv, bv):
    res = _run(_in_maps(x, Wq, bq, Wk, bk, Wv, bv))
    out = np.concatenate(
        [np.concatenate([res.results[c]["y0"], res.results[c]["y1"]], axis=1) for c in range(NCORES)],
        axis=1,
    )
    return out[None].astype(np.float32)


def bench(x, Wq, bq, Wk, bk, Wv, bv):
    """Run with NTFF tracing; returns (output, exec_time_ns)."""
    res = _run(_in_maps(x, Wq, bq, Wk, bk, Wv, bv), trace=True)
    out = np.concatenate(
        [np.concatenate([res.results[c]["y0"], res.results[c]["y1"]], axis=1) for c in range(NCORES)],
        axis=1,
    )
    return out[None].astype(np.float32), res.exec_time_ns
